# revision 2
# baseline (speedup 1.0000x reference)
"""Trainium2 Bass kernel for nn_DfOpCoefLoop (deep-filter complex FIR + alpha blend).

Reference semantics (per batch b, time t, freq bin f < 96):
    spec_f[t,f] = sum_{i=0..4} x[t+i-2, f] * coefs[t,i,f]      (complex MAC, zero-padded in t)
    out[t,f]    = alpha[t] * spec_f[t,f] + (1-alpha[t]) * x[t,f]
    out[t,f]    = spec[t,f]                                    (f >= 96 passthrough)

The end-to-end wall clock is dominated by the host<->device tunnel (~76 MB/s),
so the kernel is designed to minimize transferred bytes:
  - x (the 96 deep-filtered bins) ships ONCE as fp16 [bpc, 1028, 192] with the
    time padding baked in; the 5 filter taps are re-materialized on-device by
    5 overlapping row-shifted DMAs per time chunk (HBM re-reads are free).
  - coefs ship as int8 [bpc, 1024, 960] in their NATIVE (t, order, f, c) row
    order, quantized per (b, t) row; the dequant scale is folded into the
    alpha table on the host, so dequantization costs zero device ops.
  - the blended output returns as fp16 [bpc, 1024, 192] raw rows.
  - the f >= 96 bins never touch the device (host passthrough copy).
Measured max rel-err of this scheme vs the fp32 reference: ~7e-3 (gate 2e-2).

Device program per (batch, 128-row time chunk), t = partition, (i,f,c) = free:
    X5[p, i*192:(i+1)*192] <- xh[b, 128k+i+p, :]   5 overlapping DMAs (taps)
    C8 <- ch[b, 128k+p, :]                          1 DMA, int8, native layout
    p1 = [xr*cr | -(xi*ci)]  p2 = [xi*cr | xr*ci]  (stride-2 views, fp32 out)
    re/im = 10-tap tensor_reduce of p1/p2          (DVE)
    out = (alpha*scale)*acc + (1-alpha)*x0         (per-partition scalar STT)
"""

import numpy as np
from concurrent.futures import ThreadPoolExecutor

ORDER = 5
LOOKAHEAD = 2
F = 96             # deep-filtered bins
FC = 2 * F         # one t-row of interleaved (f, c) data: 192 floats
W = ORDER * FC     # one t-row of taps / coefs: 960
NFREQ = 481
B, T = 32, 1000
NCORES = 8
BPC = B // NCORES  # batches per core
NK = 8             # 128-row time chunks per batch
TP = NK * 128      # padded time extent (1024)
XROWS = TP + ORDER - 1  # 1028: padded x rows (x[t] lives at row t+LOOKAHEAD)

_CACHE = {}
_POOL = ThreadPoolExecutor(max_workers=16)


def _build_program(bpc=BPC):
    """Per-core Bass program (compiled Bacc)."""
    import concourse.bacc as bacc
    import concourse.mybir as mybir
    import concourse.tile as tile

    ncols = bpc * NK
    nc = bacc.Bacc("TRN2", target_bir_lowering=False, debug=False)
    f16 = mybir.dt.float16
    f32 = mybir.dt.float32
    i8 = mybir.dt.int8

    xh_t = nc.dram_tensor("xh", [bpc, XROWS, FC], f16, kind="ExternalInput").ap()
    ch_t = nc.dram_tensor("ch", [bpc, TP, W], i8, kind="ExternalInput").ap()
    asc_t = nc.dram_tensor("asc_t", [128, ncols], f32, kind="ExternalInput").ap()
    oma_t = nc.dram_tensor("oma_t", [128, ncols], f32, kind="ExternalInput").ap()
    outb = nc.dram_tensor("outb", [bpc, TP * FC], f16, kind="ExternalOutput").ap()

    mul = mybir.AluOpType.mult
    add = mybir.AluOpType.add
    copy_fn = mybir.ActivationFunctionType.Copy

    with tile.TileContext(nc) as tc:
        with (
            tc.tile_pool(name="const", bufs=1) as const_pool,
            tc.tile_pool(name="x5p", bufs=3) as x5_pool,
            tc.tile_pool(name="c8p", bufs=3) as c8_pool,
            tc.tile_pool(name="p1p", bufs=2) as p1_pool,
            tc.tile_pool(name="p2p", bufs=2) as p2_pool,
            tc.tile_pool(name="accp", bufs=3) as acc_pool,
            tc.tile_pool(name="obp", bufs=2) as ob_pool,
        ):
            asc_sb = const_pool.tile([128, ncols], f32, name="asc_sb")
            oma_sb = const_pool.tile([128, ncols], f32, name="oma_sb")
            nc.sync.dma_start(asc_sb[:], asc_t[:])
            nc.sync.dma_start(oma_sb[:], oma_t[:])

            for b in range(bpc):
                ob = ob_pool.tile([128, NK * FC], f16, name="ob")
                for k in range(NK):
                    col = b * NK + k
                    r0 = 128 * k
                    x5 = x5_pool.tile([128, W], f16, name="x5")
                    c8 = c8_pool.tile([128, W], i8, name="c8")
                    for i in range(ORDER):
                        nc.sync.dma_start(
                            x5[:, i * FC : (i + 1) * FC],
                            xh_t[b, r0 + i : r0 + i + 128, :],
                        )
                    nc.scalar.dma_start(c8[:], ch_t[b, r0 : r0 + 128, :])

                    xv = x5[:].rearrange("p (i f c) -> p i f c", i=ORDER, f=F, c=2)
                    cv = c8[:].rearrange("p (i f c) -> p i f c", i=ORDER, f=F, c=2)
                    p1 = p1_pool.tile([128, W], f32, name="p1")
                    p2 = p2_pool.tile([128, W], f32, name="p2")
                    HB = ORDER * F  # 480

                    def half(t, h):
                        return t[:, h * HB : (h + 1) * HB].rearrange(
                            "p (i f) -> p i f", i=ORDER, f=F
                        )

                    # p1 = [xr*cr | -(xi*ci)] ; p2 = [xi*cr | xr*ci]
                    nc.gpsimd.tensor_mul(half(p1, 0), xv[:, :, :, 0], cv[:, :, :, 0])
                    nc.vector.scalar_tensor_tensor(
                        half(p1, 1), xv[:, :, :, 1], -1.0, cv[:, :, :, 1],
                        op0=mul, op1=mul,
                    )
                    nc.gpsimd.tensor_mul(half(p2, 0), xv[:, :, :, 1], cv[:, :, :, 0])
                    nc.gpsimd.tensor_mul(half(p2, 1), xv[:, :, :, 0], cv[:, :, :, 1])

                    acc = acc_pool.tile([128, FC], f32, name="acc")
                    v = acc_pool.tile([128, FC], f32, name="v")
                    nc.vector.tensor_reduce(
                        acc[:, 0:F],
                        p1[:].rearrange("p (j f) -> p f j", j=2 * ORDER, f=F),
                        axis=mybir.AxisListType.X,
                        op=add,
                    )
                    nc.vector.tensor_reduce(
                        acc[:, F:FC],
                        p2[:].rearrange("p (j f) -> p f j", j=2 * ORDER, f=F),
                        axis=mybir.AxisListType.X,
                        op=add,
                    )
                    # v = (1-alpha) * x0  (x0 = center tap, interleaved (f,c))
                    nc.scalar.activation(
                        v[:],
                        x5[:, LOOKAHEAD * FC : (LOOKAHEAD + 1) * FC],
                        copy_fn,
                        scale=oma_sb[:, col : col + 1],
                    )
                    # out = (alpha*qscale)*acc + v   (acc planar -> interleaved)
                    nc.vector.scalar_tensor_tensor(
                        ob[:, k * FC : (k + 1) * FC].rearrange(
                            "p (f c) -> p f c", f=F, c=2
                        ),
                        acc[:].rearrange("p (c f) -> p f c", c=2, f=F),
                        asc_sb[:, col : col + 1],
                        v[:].rearrange("p (f c) -> p f c", f=F, c=2),
                        op0=mul,
                        op1=add,
                    )
                nc.sync.dma_start(
                    outb[b].rearrange("(k p f) -> p k f", k=NK, p=128, f=FC),
                    ob[:].rearrange("p (k f) -> p k f", k=NK, f=FC),
                )
    nc.compile()
    return nc


def _get_program():
    if "prog" not in _CACHE:
        _CACHE["prog"] = _build_program()
    return _CACHE["prog"]


def _host_prep(spec, coefs, alpha):
    """Quantize + lay out all inputs (threaded over batches)."""
    xh = np.zeros((B, XROWS, FC), np.float16)
    ch = np.zeros((B, TP, W), np.int8)
    asc = np.zeros((B, TP), np.float32)
    oma = np.zeros((B, TP), np.float32)

    def work(b):
        xh[b, LOOKAHEAD : LOOKAHEAD + T].reshape(T, F, 2)[...] = spec[b, 0, :, :F, :]
        cb = coefs[b].reshape(T, W)
        m = np.abs(cb).max(axis=1)
        np.maximum(m, np.float32(1e-20), out=m)
        q = cb * (np.float32(127.0) / m)[:, None]
        np.rint(q, out=q)
        np.clip(q, -127, 127, out=q)
        ch[b, :T] = q
        a = alpha[b, :, 0]
        asc[b, :T] = a * (m * np.float32(1.0 / 127.0))
        oma[b, :T] = np.float32(1.0) - a

    list(_POOL.map(work, range(B)))

    def table(a, c):
        # (bpc, TP) -> [128, bpc*NK] with partition = t%128, col = b*NK + k
        return np.ascontiguousarray(
            a[c * BPC : (c + 1) * BPC]
            .reshape(BPC, NK, 128)
            .transpose(2, 0, 1)
            .reshape(128, BPC * NK)
        )

    return [
        {
            "xh": xh[c * BPC : (c + 1) * BPC],
            "ch": ch[c * BPC : (c + 1) * BPC],
            "asc_t": table(asc, c),
            "oma_t": table(oma, c),
        }
        for c in range(NCORES)
    ]


def run_on_cores(spec, coefs, alpha, trace=False):
    """Full-input entry: shard, run on 8 cores, return (out_full, results_obj)."""
    from concourse import bass_utils

    nc = _get_program()
    in_maps = _host_prep(spec, coefs, alpha)
    res = bass_utils.run_bass_kernel_spmd(
        nc, in_maps, core_ids=list(range(NCORES)), trace=trace
    )

    full = np.array(spec, dtype=np.float32, copy=True)  # f>=96 passthrough on host

    def fill(c):
        ob = res.results[c]["outb"]  # (bpc, TP*FC) fp16
        full[c * BPC : (c + 1) * BPC, 0, :, :F, :] = ob.reshape(BPC, TP, F, 2)[
            :, :T
        ]

    list(_POOL.map(fill, range(NCORES)))
    return full, res


def kernel(spec, coefs, alpha):
    spec = np.asarray(spec, dtype=np.float32)
    coefs = np.asarray(coefs, dtype=np.float32)
    alpha = np.asarray(alpha, dtype=np.float32)
    full, _ = run_on_cores(spec, coefs, alpha, trace=False)
    return full
